# revision 25
# baseline (speedup 1.0000x reference)
"""Trainium2 Bass kernel for nn_Aggregation0 (scatter_memory).

8 cores = 4 frames x 2 image-halves (SPMD, one program). The host pre-sorts
patch rows into destination order per core (the sharding/permutation step);
the device does the full aggregation compute with sequential DMA:
  fold: bf16 matmuls vs shifted-identity weights accumulate DIRECTLY into
  per-block row ribbons in PSUM (dst AP aliases columns at 3*(g+i)+c, a
  zero-init matmul arms the has_written bits so aliased writes accumulate)
  -> DVE seam-join + 1/w normalize (cast to bf16)
  -> bf16 PE transpose to (row,ch)-major -> transpose-mode unfold per column
  shift -> strided assembly copies (bf16) -> sequential store (host
  inverse-maps). All HBM traffic is bf16 (tolerance 2e-2 >> bf16 roundoff).
"""
import sys
if '/opt/trn_rl_repo' not in sys.path:
    sys.path.insert(0, '/opt/trn_rl_repo')
import numpy as np

import concourse.bacc as bacc
import concourse.bass as bass
import concourse.mybir as mybir
import concourse.tile as tile
from concourse.bass_utils import run_bass_kernel_spmd

T, HP, WP = 4, 256, 256
PS = 7
NPOS = 250
P = NPOS * NPOS
B = 24                   # tops per block
NB = 6
BT = [B] * 5 + [11]
RC = 3 * (B + 6)         # rowtile (r,ch) columns = 90
GW = 131
NREAL = GW * NPOS        # 32750
F32 = mybir.dt.float32
BF16 = mybir.dt.bfloat16
D = 147
# out-tops per block k: [Bk-6, Bk+B-7] clipped to [0, 130]
OT = [(max(0, B * k - 6), min(130, B * k + B - 7)) for k in range(NB)]
SIN = sum(BT) * 2            # 262 input slots of [128, 147]
SOUT = sum(t[1] - t[0] + 1 for t in OT) * 2   # 262 output slots
IN_W = SIN * D
OUT_W = SOUT * D


# device rows use (j, i, ch) element order; COLPERM maps back to (ch, i, j)
COLPERM = np.zeros(147, np.int64)
for _c in range(3):
    for _i in range(7):
        for _j in range(7):
            COLPERM[_c * 49 + _i * 7 + _j] = _j * 21 + _i * 3 + _c


def _cntf(z):
    z = np.asarray(z, np.float64)
    return np.minimum(6, z) - np.maximum(0, z - 249) + 1


def _host_prep_core(x, nlInds, c):
    f, h = c >> 1, c & 1
    g0 = 0 if h == 0 else 119
    o_lo, o_hi = (0, 124) if h == 0 else (6, 130)
    inds = nlInds[f, :, 0]
    top = inds[:, 1].astype(np.int64)
    left = inds[:, 2].astype(np.int64)
    invperm = np.empty(P, np.int64)
    invperm[top * NPOS + left] = np.arange(P)
    sel = np.nonzero((top >= g0) & (top <= g0 + 130))[0]
    rank = np.full(P, -1, np.int64)
    rank[sel] = np.arange(NREAL)
    ar128 = np.arange(128)
    ZR, DM = NREAL, -1
    gidx = np.zeros((SIN, 128), np.int64)
    s = 0
    for k in range(NB):
        for g in range(BT[k]):
            gt_ = g0 + B * k + g
            for ci, base in ((0, 0), (1, 122)):
                gidx[s] = rank[invperm[gt_ * NPOS + base + ar128]]
                s += 1
    sidx = np.full((SOUT, 128), DM, np.int64)
    s = 0
    for k in range(NB):
        t_lo, t_hi = OT[k]
        # ci-major slot order (matches device stg layout / half-DMA split)
        for ci, base in ((0, 0), (1, 122)):
            for lt in range(t_lo, t_hi + 1):
                gt_ = g0 + lt
                if o_lo <= lt <= o_hi:
                    col = rank[invperm[gt_ * NPOS + base + ar128]]
                    sidx[s] = col
                    if ci == 1:
                        sidx[s, :6] = DM
                s += 1
    # x plane: (128, SIN*147) dest-ordered slots, single bf16 plane
    # (rel-err tolerance 2e-2 >> bf16 roundoff ~4e-3)
    import ml_dtypes
    xp = np.concatenate([x[f, sel, 0], np.zeros((1, D), np.float32)], axis=0)
    xs = xp[gidx.reshape(-1)].reshape(SIN, 128, D).transpose(1, 0, 2) \
        .reshape(128, IN_W)
    # reorder each block's slab (g, ci, c, i, j) -> (ci, j, g, c, i) so the
    # device fold matmul's rhs stream is contiguous per (chunk, j)
    off = 0
    for k in range(NB):
        G = BT[k]
        w = G * 2 * D
        slab = xs[:, off:off + w].reshape(128, G, 2, 3, 7, 7)
        xs[:, off:off + w] = np.ascontiguousarray(
            slab.transpose(0, 2, 5, 1, 3, 4)).reshape(128, w)
        off += w
    x_hi = np.ascontiguousarray(xs.astype(ml_dtypes.bfloat16))
    # winv per (block, chunk): [128(par c), 90(r,ch)]
    winv = np.zeros((NB, 2, 128, RC), np.float32)
    for k in range(NB):
        for rl in range(B + 6):
            gr = g0 + B * k - 6 + rl
            lr = B * k - 6 + rl
            if 0 <= gr <= 255 and 0 <= lr <= 136:
                wr = _cntf(gr)
                for chunk in range(2):
                    cs = chunk * 128 + np.arange(128)
                    winv[k, chunk, :, rl * 3:rl * 3 + 3] = \
                        (1.0 / (wr * _cntf(cs)))[:, None]
    return dict(x_hi=x_hi,
                winv=np.ascontiguousarray(
                    winv.transpose(2, 0, 1, 3).reshape(128, NB * 2 * RC)),
                f=f, sel=sel, sidx=sidx)


def _identw():
    w = np.zeros((128, 262), np.float32)
    w[np.arange(128), np.arange(128) + 128] = 1.0
    return w


def _ap(base, off, dims):
    return bass.AP(base.tensor, base.offset + off,
                   [list(base.ap[0])] + [list(d) for d in dims])


def build_nc():
    nc = bacc.Bacc("TRN2", target_bir_lowering=False, debug=False, num_devices=8)
    xh_d = nc.declare_dram_parameter("x_hi", [128, IN_W], BF16, isOutput=False)
    ib_d = nc.declare_dram_parameter("identb", [128, 262], BF16, isOutput=False)
    wv_d = nc.declare_dram_parameter("winv", [128, NB * 2 * RC], F32,
                                     isOutput=False)
    y_d = nc.declare_dram_parameter("y_core", [128, OUT_W], BF16, isOutput=True)

    ADD = mybir.AluOpType.add
    MUL = mybir.AluOpType.mult

    with tile.TileContext(nc) as tc:
        with tc.tile_pool(name="const", bufs=1) as cpool, \
             tc.tile_pool(name="gp", bufs=3) as gpool, \
             tc.tile_pool(name="rps", bufs=2, space="PSUM") as rps, \
             tc.tile_pool(name="rbs", bufs=3) as rbsp, \
             tc.tile_pool(name="vtsb", bufs=2) as vtsbp, \
             tc.tile_pool(name="vtbp", bufs=3) as vtbp, \
             tc.tile_pool(name="vbp", bufs=1, space="PSUM") as vbp, \
             tc.tile_pool(name="vbs", bufs=2) as vbs, \
             tc.tile_pool(name="pjp", bufs=3, space="PSUM") as pjp, \
             tc.tile_pool(name="stg", bufs=3) as stgp:
            # block 0's chunk-0 half is issued before the consts so the
            # first fold can start ~3us in; each block's input lands as two
            # per-chunk half-DMAs (chunk-0 folds depend only on the first)
            G0D = BT[0] * D
            gth0 = gpool.tile([128, 2 * G0D], BF16, tag="gth", name="gth0")
            nc.sync.dma_start(out=gth0[:, 0:G0D], in_=xh_d[:, 0:G0D])
            identb = cpool.tile([128, 262], BF16)
            nc.sync.dma_start(out=identb[:], in_=ib_d[:])
            nc.sync.dma_start(out=gth0[:, G0D:2 * G0D],
                              in_=xh_d[:, G0D:2 * G0D])
            wvt = cpool.tile([128, NB * 2 * RC], F32)
            nc.sync.dma_start(out=wvt[:], in_=wv_d[:])

            rbs_hist = [None] * NB
            vtA_hist = [None] * NB
            vtb_hist = [None] * NB
            in_off = [2 * G0D]
            out_off = [0]

            def fold_block(k):
                G = BT[k]
                w = G * 2 * D
                if k == 0:
                    gth = gth0
                else:
                    gth = gpool.tile([128, w], BF16, tag="gth",
                                     name=f"gth{k}")
                    half = G * D
                    nc.sync.dma_start(
                        out=gth[:, 0:half],
                        in_=xh_d[:, in_off[0]: in_off[0] + half])
                    nc.sync.dma_start(
                        out=gth[:, half:w],
                        in_=xh_d[:, in_off[0] + half: in_off[0] + w])
                    in_off[0] += w
                rbss = []
                for chunk in range(2):
                    rb = rps.tile([128, G * 21], F32, tag=f"ribs{chunk}",
                                  name=f"ribs{chunk}_{k}")
                    for j in range(7):
                        d = j if chunk == 0 else j - 6
                        # host stores each block j-plane-major, so the rhs
                        # stream is fully contiguous (fast PE ifmap fetch)
                        rhs = _ap(gth[:], (chunk * 7 + j) * G * 21,
                                  [(1, G * 21)])
                        nc.tensor.matmul(
                            rb[:], lhsT=identb[:, 128 - d:256 - d],
                            rhs=rhs, start=(j == 0), stop=(j == 6))
                    # stage ribbons to SBUF (bf16) so the row-fold adds can
                    # run on DVE/GpSimd without the PSUM port bottleneck
                    rbst = rbsp.tile([128, G * 21], BF16, tag=f"rbs{chunk}",
                                     name=f"rbs{chunk}_{k}")
                    nc.scalar.copy(out=rbst[:], in_=rb[:])
                    rbss.append(rbst)
                rbs_hist[k] = rbss
                # role-1 partial row-fold (rows Bk-6..Bk+5, window cols
                # [0,36)) reads the PREVIOUS block's ribbons — issued here
                # so DVE/GpSimd execute it while the PE is still folding
                if k >= 1:
                    Gg = BT[k - 1]
                    vtAs = []
                    for chunk in range(2):
                        eng = nc.vector if chunk == 0 else nc.gpsimd
                        vtA = vtsbp.tile([128, 36], F32, tag=f"vtA{chunk}",
                                         name=f"vtA{chunk}_{k}")
                        for i in range(6, -1, -1):   # i=6 covers [0,36)
                            g_lo = 18 - i
                            ng = Gg + i - 18
                            dst = _ap(vtA[:], 0, [(3, ng), (1, 3)])
                            src = _ap(rbs_hist[k - 1][chunk][:],
                                      g_lo * 21 + i, [(21, ng), (7, 3)])
                            if i == 6:
                                eng.tensor_copy(out=dst, in_=src)
                            else:
                                eng.tensor_tensor(out=dst, in0=dst, in1=src,
                                                  op=ADD)
                        vtAs.append(vtA)
                    vtA_hist[k] = vtAs
                    rbs_hist[k - 1] = None

            def vt_stage(k):
                G = BT[k]
                vtbs = []
                for chunk in range(2):
                    # chunk 0 folds on DVE, chunk 1 on GpSimd (idle engine,
                    # SBUF-only — that's why ribbons were staged to SBUF)
                    eng = nc.vector if chunk == 0 else nc.gpsimd
                    # role-0: gen k, rows Bk..Bk+23 = window cols [18,90),
                    # accumulated in a 72-wide tile (window col - 18)
                    vtB = vtsbp.tile([128, 72], F32, tag=f"vtB{chunk}",
                                     name=f"vtB{chunk}_{k}")
                    if G < B:
                        eng.memset(vtB[:], 0.0)
                    for i in range(7):
                        ng = min(G - 1, 23 - i) + 1
                        dst = _ap(vtB[:], i * 3, [(3, ng), (1, 3)])
                        src = _ap(rbs_hist[k][chunk][:], i, [(21, ng), (7, 3)])
                        if i == 0 and G == B:   # i=0 covers [0,72)
                            eng.tensor_copy(out=dst, in_=src)
                        else:
                            eng.tensor_tensor(out=dst, in0=dst, in1=src,
                                              op=ADD)
                    vtb = vtbp.tile([128, RC], BF16, tag=f"vtb{chunk}",
                                    name=f"vtb{chunk}_{k}")
                    wv = wvt[:, (k * 2 + chunk) * RC:(k * 2 + chunk + 1) * RC]
                    if k == 0:
                        eng.memset(vtb[:, 0:18], 0.0)
                        eng.tensor_tensor(out=vtb[:, 18:90], in0=vtB[:],
                                          in1=wv[:, 18:90], op=MUL)
                    else:
                        vtA = vtA_hist[k][chunk]
                        eng.tensor_tensor(out=vtb[:, 0:18], in0=vtA[:, 0:18],
                                          in1=wv[:, 0:18], op=MUL)
                        eng.tensor_tensor(out=vtA[:, 18:36],
                                          in0=vtA[:, 18:36],
                                          in1=vtB[:, 0:18], op=ADD)
                        eng.tensor_tensor(out=vtb[:, 18:36],
                                          in0=vtA[:, 18:36],
                                          in1=wv[:, 18:36], op=MUL)
                        eng.tensor_tensor(out=vtb[:, 36:90],
                                          in0=vtB[:, 18:72],
                                          in1=wv[:, 36:90], op=MUL)
                    vtbs.append(vtb)
                vtb_hist[k] = vtbs

            def unfold_tail(k):
                vtbs = vtb_hist[k]
                vb = vbp.tile([RC, 256], BF16, tag="vb", name=f"vb{k}")
                for chunk in range(2):
                    nc.tensor.matmul(
                        vb[:, chunk * 128:(chunk + 1) * 128],
                        lhsT=vtbs[chunk][:, 0:RC], rhs=identb[:, 128:256],
                        is_transpose=True,
                        start=(chunk == 0), stop=(chunk == 1))
                vsb = vbs.tile([RC, 256], BF16, tag="vsb", name=f"vsb{k}")
                nc.scalar.copy(out=vsb[:], in_=vb[:])
                t_lo, t_hi = OT[k]
                nt = t_hi - t_lo + 1
                goff = t_lo - (B * k - 6)
                w = nt * 2 * D
                half = nt * D
                stg = stgp.tile([128, w], BF16, tag="stg", name=f"stg{k}")
                cpy = 0
                # ci-major stg layout: each ci half stores as soon as its 7
                # copies land. Stores go via gpsimd SWDGE: keeps them off
                # the Sync HWDGE FIFO so the next block's input load isn't
                # head-of-line blocked behind this store's dependencies
                for ci, base in ((0, 0), (1, 122)):
                    for j in range(7):
                        pj = pjp.tile([128, RC], BF16, tag="pj",
                                      name=f"pj{k}_{j}_{ci}")
                        nc.tensor.matmul(
                            pj[:], lhsT=vsb[:, base + j: base + j + 128],
                            rhs=identb[0:RC, 128:128 + RC],
                            is_transpose=True, start=True, stop=True)
                        src = _ap(pj[:], goff * 3, [(3, nt), (3, 7), (1, 3)])
                        dst = _ap(stg[:], ci * half + j * 21,
                                  [(D, nt), (1, 21)])
                        if cpy % 2 == 0:
                            nc.scalar.copy(out=dst, in_=src)
                        else:
                            nc.vector.tensor_copy(out=dst, in_=src)
                        cpy += 1
                    nc.gpsimd.dma_start(
                        out=y_d[:, out_off[0] + ci * half:
                                out_off[0] + (ci + 1) * half],
                        in_=stg[:, ci * half:(ci + 1) * half])
                out_off[0] += w

            for k in range(NB):
                fold_block(k)
                vt_stage(k)
                unfold_tail(k)

    nc.compile()
    return nc


_NC_CACHE = [None]


def _build_in_maps(x, nlInds):
    cores = [_host_prep_core(x, nlInds, c) for c in range(8)]
    idw = _identw()
    import ml_dtypes
    idb = idw.astype(ml_dtypes.bfloat16)
    in_maps = [dict(x_hi=cr["x_hi"], winv=cr["winv"],
                    identb=idb) for cr in cores]
    return cores, in_maps


def kernel(x, nlDists, nlInds, pixels_h, pixels_w):
    x = np.ascontiguousarray(np.asarray(x, dtype=np.float32))
    nlInds = np.asarray(nlInds)
    if _NC_CACHE[0] is None:
        _NC_CACHE[0] = build_nc()
    nc = _NC_CACHE[0]
    cores, in_maps = _build_in_maps(x, nlInds)
    res = run_bass_kernel_spmd(nc, in_maps, list(range(8)))
    out = np.zeros((T, P, 1, 147), np.float32)
    for c in range(8):
        cr = cores[c]
        y = np.asarray(res.results[c]["y_core"], np.float32)  # (128, OUT_W)
        ys = y.reshape(128, SOUT, D).transpose(1, 0, 2).reshape(-1, D)
        sidx = cr["sidx"].reshape(-1)
        valid = sidx >= 0
        out[cr["f"], cr["sel"][sidx[valid]], 0] = ys[valid][:, COLPERM]
    return out


# revision 26
# speedup vs baseline: 1.0518x; 1.0518x over previous
"""Trainium2 Bass kernel for nn_Aggregation0 (scatter_memory).

8 cores = 4 frames x 2 image-halves (SPMD, one program). The host pre-sorts
patch rows into destination order per core (the sharding/permutation step);
the device does the full aggregation compute with sequential DMA:
  fold: bf16 matmuls vs shifted-identity weights accumulate DIRECTLY into
  per-block row ribbons in PSUM (dst AP aliases columns at 3*(g+i)+c, a
  zero-init matmul arms the has_written bits so aliased writes accumulate)
  -> DVE seam-join + 1/w normalize (cast to bf16)
  -> bf16 PE transpose to (row,ch)-major -> transpose-mode unfold per column
  shift -> strided assembly copies (bf16) -> sequential store (host
  inverse-maps). All HBM traffic is bf16 (tolerance 2e-2 >> bf16 roundoff).
"""
import sys
if '/opt/trn_rl_repo' not in sys.path:
    sys.path.insert(0, '/opt/trn_rl_repo')
import numpy as np

import concourse.bacc as bacc
import concourse.bass as bass
import concourse.mybir as mybir
import concourse.tile as tile
from concourse.bass_utils import run_bass_kernel_spmd

T, HP, WP = 4, 256, 256
PS = 7
NPOS = 250
P = NPOS * NPOS
B = 24                   # tops per block
NB = 6
BT = [B] * 5 + [11]
RC = 3 * (B + 6)         # rowtile (r,ch) columns = 90
GW = 131
NREAL = GW * NPOS        # 32750
F32 = mybir.dt.float32
BF16 = mybir.dt.bfloat16
D = 147
# out-tops per block k: [Bk-6, Bk+B-7] clipped to [0, 130]
OT = [(max(0, B * k - 6), min(130, B * k + B - 7)) for k in range(NB)]
SIN = sum(BT) * 2            # 262 input slots of [128, 147]
SOUT = sum(t[1] - t[0] + 1 for t in OT) * 2   # 262 output slots
IN_W = SIN * D
OUT_W = SOUT * D


# device rows use (j, i, ch) element order; COLPERM maps back to (ch, i, j)
COLPERM = np.zeros(147, np.int64)
for _c in range(3):
    for _i in range(7):
        for _j in range(7):
            COLPERM[_c * 49 + _i * 7 + _j] = _j * 21 + _i * 3 + _c


def _cntf(z):
    z = np.asarray(z, np.float64)
    return np.minimum(6, z) - np.maximum(0, z - 249) + 1


def _host_prep_core(x, nlInds, c):
    f, h = c >> 1, c & 1
    g0 = 0 if h == 0 else 119
    o_lo, o_hi = (0, 124) if h == 0 else (6, 130)
    inds = nlInds[f, :, 0]
    top = inds[:, 1].astype(np.int64)
    left = inds[:, 2].astype(np.int64)
    invperm = np.empty(P, np.int64)
    invperm[top * NPOS + left] = np.arange(P)
    sel = np.nonzero((top >= g0) & (top <= g0 + 130))[0]
    rank = np.full(P, -1, np.int64)
    rank[sel] = np.arange(NREAL)
    ar128 = np.arange(128)
    ZR, DM = NREAL, -1
    gidx = np.zeros((SIN, 128), np.int64)
    s = 0
    for k in range(NB):
        for g in range(BT[k]):
            gt_ = g0 + B * k + g
            for ci, base in ((0, 0), (1, 122)):
                gidx[s] = rank[invperm[gt_ * NPOS + base + ar128]]
                s += 1
    sidx = np.full((SOUT, 128), DM, np.int64)
    s = 0
    for k in range(NB):
        t_lo, t_hi = OT[k]
        # ci-major slot order (matches device stg layout / half-DMA split)
        for ci, base in ((0, 0), (1, 122)):
            for lt in range(t_lo, t_hi + 1):
                gt_ = g0 + lt
                if o_lo <= lt <= o_hi:
                    col = rank[invperm[gt_ * NPOS + base + ar128]]
                    sidx[s] = col
                    if ci == 1:
                        sidx[s, :6] = DM
                s += 1
    # x plane: (128, SIN*147) dest-ordered slots, single bf16 plane
    # (rel-err tolerance 2e-2 >> bf16 roundoff ~4e-3)
    import ml_dtypes
    xp = np.concatenate([x[f, sel, 0], np.zeros((1, D), np.float32)], axis=0)
    xs = xp[gidx.reshape(-1)].reshape(SIN, 128, D).transpose(1, 0, 2) \
        .reshape(128, IN_W)
    # reorder each block's slab (g, ci, c, i, j) -> (ci, j, g, c, i) so the
    # device fold matmul's rhs stream is contiguous per (chunk, j)
    off = 0
    for k in range(NB):
        G = BT[k]
        w = G * 2 * D
        slab = xs[:, off:off + w].reshape(128, G, 2, 3, 7, 7)
        xs[:, off:off + w] = np.ascontiguousarray(
            slab.transpose(0, 2, 5, 1, 3, 4)).reshape(128, w)
        off += w
    x_hi = np.ascontiguousarray(xs.astype(ml_dtypes.bfloat16))
    # winv per (block, chunk): [128(par c), 90(r,ch)]
    winv = np.zeros((NB, 2, 128, RC), np.float32)
    for k in range(NB):
        for rl in range(B + 6):
            gr = g0 + B * k - 6 + rl
            lr = B * k - 6 + rl
            if 0 <= gr <= 255 and 0 <= lr <= 136:
                wr = _cntf(gr)
                for chunk in range(2):
                    cs = chunk * 128 + np.arange(128)
                    winv[k, chunk, :, rl * 3:rl * 3 + 3] = \
                        (1.0 / (wr * _cntf(cs)))[:, None]
    return dict(x_hi=x_hi,
                winv=np.ascontiguousarray(
                    winv.transpose(2, 0, 1, 3).reshape(128, NB * 2 * RC)),
                f=f, sel=sel, sidx=sidx)


def _identw():
    w = np.zeros((128, 262), np.float32)
    w[np.arange(128), np.arange(128) + 128] = 1.0
    return w


def _ap(base, off, dims):
    return bass.AP(base.tensor, base.offset + off,
                   [list(base.ap[0])] + [list(d) for d in dims])


def build_nc():
    nc = bacc.Bacc("TRN2", target_bir_lowering=False, debug=False, num_devices=8)
    xh_d = nc.declare_dram_parameter("x_hi", [128, IN_W], BF16, isOutput=False)
    ib_d = nc.declare_dram_parameter("identb", [128, 262], BF16, isOutput=False)
    wv_d = nc.declare_dram_parameter("winv", [128, NB * 2 * RC], F32,
                                     isOutput=False)
    y_d = nc.declare_dram_parameter("y_core", [128, OUT_W], BF16, isOutput=True)

    ADD = mybir.AluOpType.add
    MUL = mybir.AluOpType.mult

    with tile.TileContext(nc) as tc:
        with tc.tile_pool(name="const", bufs=1) as cpool, \
             tc.tile_pool(name="gp", bufs=3) as gpool, \
             tc.tile_pool(name="rps", bufs=2, space="PSUM") as rps, \
             tc.tile_pool(name="rbs", bufs=3) as rbsp, \
             tc.tile_pool(name="vtsb", bufs=2) as vtsbp, \
             tc.tile_pool(name="vtbp", bufs=3) as vtbp, \
             tc.tile_pool(name="vbp", bufs=1, space="PSUM") as vbp, \
             tc.tile_pool(name="vbs", bufs=2) as vbs, \
             tc.tile_pool(name="pjp", bufs=3, space="PSUM") as pjp, \
             tc.tile_pool(name="stg", bufs=3) as stgp:
            # block 0's chunk-0 half is issued before the consts so the
            # first fold can start ~3us in; each block's input lands as two
            # per-chunk half-DMAs (chunk-0 folds depend only on the first)
            G0D = BT[0] * D
            gth0 = gpool.tile([128, 2 * G0D], BF16, tag="gth", name="gth0")
            nc.sync.dma_start(out=gth0[:, 0:G0D], in_=xh_d[:, 0:G0D])
            identb = cpool.tile([128, 262], BF16)
            nc.sync.dma_start(out=identb[:], in_=ib_d[:])
            nc.sync.dma_start(out=gth0[:, G0D:2 * G0D],
                              in_=xh_d[:, G0D:2 * G0D])
            wvt = cpool.tile([128, NB * 2 * RC], F32)
            nc.sync.dma_start(out=wvt[:], in_=wv_d[:])

            rbs_hist = [None] * NB
            vtA_hist = [None] * NB
            vtb_hist = [None] * NB
            in_off = [2 * G0D]
            out_off = [0]

            def fold_block(k):
                G = BT[k]
                w = G * 2 * D
                if k == 0:
                    gth = gth0
                else:
                    gth = gpool.tile([128, w], BF16, tag="gth",
                                     name=f"gth{k}")
                    half = G * D
                    nc.sync.dma_start(
                        out=gth[:, 0:half],
                        in_=xh_d[:, in_off[0]: in_off[0] + half])
                    nc.sync.dma_start(
                        out=gth[:, half:w],
                        in_=xh_d[:, in_off[0] + half: in_off[0] + w])
                    in_off[0] += w
                rbss = []
                for chunk in range(2):
                    rb = rps.tile([128, G * 21], F32, tag=f"ribs{chunk}",
                                  name=f"ribs{chunk}_{k}")
                    for j in range(7):
                        d = j if chunk == 0 else j - 6
                        # host stores each block j-plane-major, so the rhs
                        # stream is fully contiguous (fast PE ifmap fetch)
                        rhs = _ap(gth[:], (chunk * 7 + j) * G * 21,
                                  [(1, G * 21)])
                        nc.tensor.matmul(
                            rb[:], lhsT=identb[:, 128 - d:256 - d],
                            rhs=rhs, start=(j == 0), stop=(j == 6))
                    # stage ribbons to SBUF (bf16) so the row-fold adds can
                    # run on DVE/GpSimd without the PSUM port bottleneck
                    rbst = rbsp.tile([128, G * 21], BF16, tag=f"rbs{chunk}",
                                     name=f"rbs{chunk}_{k}")
                    nc.scalar.copy(out=rbst[:], in_=rb[:])
                    rbss.append(rbst)
                rbs_hist[k] = rbss
                # role-1 partial row-fold (rows Bk-6..Bk+5, window cols
                # [0,36)) reads the PREVIOUS block's ribbons — issued here
                # so DVE/GpSimd execute it while the PE is still folding
                if k >= 1:
                    Gg = BT[k - 1]
                    vtAs = []
                    for chunk in range(2):
                        eng = nc.vector if chunk == 0 else nc.gpsimd
                        vtA = vtsbp.tile([128, 36], F32, tag=f"vtA{chunk}",
                                         name=f"vtA{chunk}_{k}")
                        for i in range(6, -1, -1):   # i=6 covers [0,36)
                            g_lo = 18 - i
                            ng = Gg + i - 18
                            dst = _ap(vtA[:], 0, [(3, ng), (1, 3)])
                            src = _ap(rbs_hist[k - 1][chunk][:],
                                      g_lo * 21 + i, [(21, ng), (7, 3)])
                            if i == 6:
                                eng.tensor_copy(out=dst, in_=src)
                            else:
                                eng.tensor_tensor(out=dst, in0=dst, in1=src,
                                                  op=ADD)
                        vtAs.append(vtA)
                    vtA_hist[k] = vtAs
                    rbs_hist[k - 1] = None

            def vt_stage(k):
                G = BT[k]
                vtbs = []
                for chunk in range(2):
                    # chunk 0 folds on DVE, chunk 1 on GpSimd (idle engine,
                    # SBUF-only — that's why ribbons were staged to SBUF)
                    eng = nc.vector if chunk == 0 else nc.gpsimd
                    # role-0: gen k, rows Bk..Bk+23 = window cols [18,90),
                    # accumulated in a 72-wide tile (window col - 18)
                    vtB = vtsbp.tile([128, 72], F32, tag=f"vtB{chunk}",
                                     name=f"vtB{chunk}_{k}")
                    if G < B:
                        eng.memset(vtB[:], 0.0)
                    for i in range(7):
                        ng = min(G - 1, 23 - i) + 1
                        dst = _ap(vtB[:], i * 3, [(3, ng), (1, 3)])
                        src = _ap(rbs_hist[k][chunk][:], i, [(21, ng), (7, 3)])
                        if i == 0 and G == B:   # i=0 covers [0,72)
                            eng.tensor_copy(out=dst, in_=src)
                        else:
                            eng.tensor_tensor(out=dst, in0=dst, in1=src,
                                              op=ADD)
                    vtb = vtbp.tile([128, RC], BF16, tag=f"vtb{chunk}",
                                    name=f"vtb{chunk}_{k}")
                    wv = wvt[:, (k * 2 + chunk) * RC:(k * 2 + chunk + 1) * RC]
                    if k == 0:
                        eng.memset(vtb[:, 0:18], 0.0)
                        eng.tensor_tensor(out=vtb[:, 18:90], in0=vtB[:],
                                          in1=wv[:, 18:90], op=MUL)
                    else:
                        vtA = vtA_hist[k][chunk]
                        eng.tensor_tensor(out=vtb[:, 0:18], in0=vtA[:, 0:18],
                                          in1=wv[:, 0:18], op=MUL)
                        eng.tensor_tensor(out=vtA[:, 18:36],
                                          in0=vtA[:, 18:36],
                                          in1=vtB[:, 0:18], op=ADD)
                        eng.tensor_tensor(out=vtb[:, 18:36],
                                          in0=vtA[:, 18:36],
                                          in1=wv[:, 18:36], op=MUL)
                        eng.tensor_tensor(out=vtb[:, 36:90],
                                          in0=vtB[:, 18:72],
                                          in1=wv[:, 36:90], op=MUL)
                    vtbs.append(vtb)
                vtb_hist[k] = vtbs

            def unfold_tail(k):
                vtbs = vtb_hist[k]
                vb = vbp.tile([RC, 256], BF16, tag="vb", name=f"vb{k}")
                for chunk in range(2):
                    nc.tensor.matmul(
                        vb[:, chunk * 128:(chunk + 1) * 128],
                        lhsT=vtbs[chunk][:, 0:RC], rhs=identb[:, 128:256],
                        is_transpose=True,
                        start=(chunk == 0), stop=(chunk == 1))
                vsb = vbs.tile([RC, 256], BF16, tag="vsb", name=f"vsb{k}")
                nc.scalar.copy(out=vsb[:], in_=vb[:])
                t_lo, t_hi = OT[k]
                nt = t_hi - t_lo + 1
                goff = t_lo - (B * k - 6)
                w = nt * 2 * D
                half = nt * D
                stg = stgp.tile([128, w], BF16, tag="stg", name=f"stg{k}")
                cpy = 0
                # ci-major stg layout: each ci half stores as soon as its 7
                # copies land. Stores go via gpsimd SWDGE: keeps them off
                # the Sync HWDGE FIFO so the next block's input load isn't
                # head-of-line blocked behind this store's dependencies
                for ci, base in ((0, 0), (1, 122)):
                    for j in range(7):
                        pj = pjp.tile([128, RC], BF16, tag="pj",
                                      name=f"pj{k}_{j}_{ci}")
                        nc.tensor.matmul(
                            pj[:], lhsT=vsb[:, base + j: base + j + 128],
                            rhs=identb[0:RC, 128:128 + RC],
                            is_transpose=True, start=True, stop=True)
                        src = _ap(pj[:], goff * 3, [(3, nt), (3, 7), (1, 3)])
                        dst = _ap(stg[:], ci * half + j * 21,
                                  [(D, nt), (1, 21)])
                        if cpy % 2 == 0:
                            nc.scalar.copy(out=dst, in_=src)
                        else:
                            nc.vector.tensor_copy(out=dst, in_=src)
                        cpy += 1
                nc.gpsimd.dma_start(out=y_d[:, out_off[0]: out_off[0] + w],
                                    in_=stg[:])
                out_off[0] += w

            for k in range(NB):
                fold_block(k)
                vt_stage(k)
                unfold_tail(k)

    nc.compile()
    return nc


_NC_CACHE = [None]


def _build_in_maps(x, nlInds):
    cores = [_host_prep_core(x, nlInds, c) for c in range(8)]
    idw = _identw()
    import ml_dtypes
    idb = idw.astype(ml_dtypes.bfloat16)
    in_maps = [dict(x_hi=cr["x_hi"], winv=cr["winv"],
                    identb=idb) for cr in cores]
    return cores, in_maps


def kernel(x, nlDists, nlInds, pixels_h, pixels_w):
    x = np.ascontiguousarray(np.asarray(x, dtype=np.float32))
    nlInds = np.asarray(nlInds)
    if _NC_CACHE[0] is None:
        _NC_CACHE[0] = build_nc()
    nc = _NC_CACHE[0]
    cores, in_maps = _build_in_maps(x, nlInds)
    res = run_bass_kernel_spmd(nc, in_maps, list(range(8)))
    out = np.zeros((T, P, 1, 147), np.float32)
    for c in range(8):
        cr = cores[c]
        y = np.asarray(res.results[c]["y_core"], np.float32)  # (128, OUT_W)
        ys = y.reshape(128, SOUT, D).transpose(1, 0, 2).reshape(-1, D)
        sidx = cr["sidx"].reshape(-1)
        valid = sidx >= 0
        out[cr["f"], cr["sel"][sidx[valid]], 0] = ys[valid][:, COLPERM]
    return out


# revision 27
# speedup vs baseline: 1.0649x; 1.0124x over previous
"""Trainium2 Bass kernel for nn_Aggregation0 (scatter_memory).

8 cores = 4 frames x 2 image-halves (SPMD, one program). The host pre-sorts
patch rows into destination order per core (the sharding/permutation step);
the device does the full aggregation compute with sequential DMA:
  fold: bf16 matmuls vs shifted-identity weights accumulate DIRECTLY into
  per-block row ribbons in PSUM (dst AP aliases columns at 3*(g+i)+c, a
  zero-init matmul arms the has_written bits so aliased writes accumulate)
  -> DVE seam-join + 1/w normalize (cast to bf16)
  -> bf16 PE transpose to (row,ch)-major -> transpose-mode unfold per column
  shift -> strided assembly copies (bf16) -> sequential store (host
  inverse-maps). All HBM traffic is bf16 (tolerance 2e-2 >> bf16 roundoff).
"""
import sys
if '/opt/trn_rl_repo' not in sys.path:
    sys.path.insert(0, '/opt/trn_rl_repo')
import numpy as np

import concourse.bacc as bacc
import concourse.bass as bass
import concourse.mybir as mybir
import concourse.tile as tile
from concourse.bass_utils import run_bass_kernel_spmd

T, HP, WP = 4, 256, 256
PS = 7
NPOS = 250
P = NPOS * NPOS
B = 24                   # tops per block
NB = 6
BT = [B] * 5 + [11]
RC = 3 * (B + 6)         # rowtile (r,ch) columns = 90
GW = 131
NREAL = GW * NPOS        # 32750
F32 = mybir.dt.float32
BF16 = mybir.dt.bfloat16
D = 147
# out-tops per block k: [Bk-6, Bk+B-7] clipped to [0, 130]
OT = [(max(0, B * k - 6), min(130, B * k + B - 7)) for k in range(NB)]
SIN = sum(BT) * 2            # 262 input slots of [128, 147]
SOUT = sum(t[1] - t[0] + 1 for t in OT) * 2   # 262 output slots
IN_W = SIN * D
OUT_W = SOUT * D


# device rows use (j, i, ch) element order; COLPERM maps back to (ch, i, j)
COLPERM = np.zeros(147, np.int64)
for _c in range(3):
    for _i in range(7):
        for _j in range(7):
            COLPERM[_c * 49 + _i * 7 + _j] = _j * 21 + _i * 3 + _c


def _cntf(z):
    z = np.asarray(z, np.float64)
    return np.minimum(6, z) - np.maximum(0, z - 249) + 1


def _host_prep_core(x, nlInds, c):
    f, h = c >> 1, c & 1
    g0 = 0 if h == 0 else 119
    o_lo, o_hi = (0, 124) if h == 0 else (6, 130)
    inds = nlInds[f, :, 0]
    top = inds[:, 1].astype(np.int64)
    left = inds[:, 2].astype(np.int64)
    invperm = np.empty(P, np.int64)
    invperm[top * NPOS + left] = np.arange(P)
    sel = np.nonzero((top >= g0) & (top <= g0 + 130))[0]
    rank = np.full(P, -1, np.int64)
    rank[sel] = np.arange(NREAL)
    ar128 = np.arange(128)
    ZR, DM = NREAL, -1
    gidx = np.zeros((SIN, 128), np.int64)
    s = 0
    for k in range(NB):
        for g in range(BT[k]):
            gt_ = g0 + B * k + g
            for ci, base in ((0, 0), (1, 122)):
                gidx[s] = rank[invperm[gt_ * NPOS + base + ar128]]
                s += 1
    sidx = np.full((SOUT, 128), DM, np.int64)
    s = 0
    for k in range(NB):
        t_lo, t_hi = OT[k]
        # ci-major slot order (matches device stg layout / half-DMA split)
        for ci, base in ((0, 0), (1, 122)):
            for lt in range(t_lo, t_hi + 1):
                gt_ = g0 + lt
                if o_lo <= lt <= o_hi:
                    col = rank[invperm[gt_ * NPOS + base + ar128]]
                    sidx[s] = col
                    if ci == 1:
                        sidx[s, :6] = DM
                s += 1
    # x plane: (128, SIN*147) dest-ordered slots, single bf16 plane
    # (rel-err tolerance 2e-2 >> bf16 roundoff ~4e-3)
    import ml_dtypes
    xp = np.concatenate([x[f, sel, 0], np.zeros((1, D), np.float32)], axis=0)
    xs = xp[gidx.reshape(-1)].reshape(SIN, 128, D).transpose(1, 0, 2) \
        .reshape(128, IN_W)
    # reorder each block's slab (g, ci, c, i, j) -> (ci, j, g, c, i) so the
    # device fold matmul's rhs stream is contiguous per (chunk, j)
    off = 0
    for k in range(NB):
        G = BT[k]
        w = G * 2 * D
        slab = xs[:, off:off + w].reshape(128, G, 2, 3, 7, 7)
        xs[:, off:off + w] = np.ascontiguousarray(
            slab.transpose(0, 2, 5, 1, 3, 4)).reshape(128, w)
        off += w
    x_hi = np.ascontiguousarray(xs.astype(ml_dtypes.bfloat16))
    # winv per (block, chunk): [128(par c), 90(r,ch)]
    winv = np.zeros((NB, 2, 128, RC), np.float32)
    for k in range(NB):
        for rl in range(B + 6):
            gr = g0 + B * k - 6 + rl
            lr = B * k - 6 + rl
            if 0 <= gr <= 255 and 0 <= lr <= 136:
                wr = _cntf(gr)
                for chunk in range(2):
                    cs = chunk * 128 + np.arange(128)
                    winv[k, chunk, :, rl * 3:rl * 3 + 3] = \
                        (1.0 / (wr * _cntf(cs)))[:, None]
    return dict(x_hi=x_hi,
                winv=np.ascontiguousarray(
                    winv.transpose(2, 0, 1, 3).reshape(128, NB * 2 * RC)),
                f=f, sel=sel, sidx=sidx)


def _identw():
    w = np.zeros((128, 262), np.float32)
    w[np.arange(128), np.arange(128) + 128] = 1.0
    return w


def _ap(base, off, dims):
    return bass.AP(base.tensor, base.offset + off,
                   [list(base.ap[0])] + [list(d) for d in dims])


def build_nc():
    nc = bacc.Bacc("TRN2", target_bir_lowering=False, debug=False, num_devices=8)
    xh_d = nc.declare_dram_parameter("x_hi", [128, IN_W], BF16, isOutput=False)
    ib_d = nc.declare_dram_parameter("identb", [128, 262], BF16, isOutput=False)
    wv_d = nc.declare_dram_parameter("winv", [128, NB * 2 * RC], F32,
                                     isOutput=False)
    y_d = nc.declare_dram_parameter("y_core", [128, OUT_W], BF16, isOutput=True)

    ADD = mybir.AluOpType.add
    MUL = mybir.AluOpType.mult

    with tile.TileContext(nc) as tc:
        with tc.tile_pool(name="const", bufs=1) as cpool, \
             tc.tile_pool(name="gp", bufs=3) as gpool, \
             tc.tile_pool(name="rps", bufs=2, space="PSUM") as rps, \
             tc.tile_pool(name="rbs", bufs=3) as rbsp, \
             tc.tile_pool(name="vtsb", bufs=2) as vtsbp, \
             tc.tile_pool(name="vtbp", bufs=3) as vtbp, \
             tc.tile_pool(name="vbp", bufs=1, space="PSUM") as vbp, \
             tc.tile_pool(name="vbs", bufs=2) as vbs, \
             tc.tile_pool(name="pjp", bufs=3, space="PSUM") as pjp, \
             tc.tile_pool(name="stg", bufs=3) as stgp:
            # block 0's chunk-0 half is issued before the consts so the
            # first fold can start ~3us in; each block's input lands as two
            # per-chunk half-DMAs (chunk-0 folds depend only on the first)
            G0D = BT[0] * D
            gth0 = gpool.tile([128, 2 * G0D], BF16, tag="gth", name="gth0")
            nc.sync.dma_start(out=gth0[:, 0:G0D], in_=xh_d[:, 0:G0D])
            identb = cpool.tile([128, 262], BF16)
            nc.sync.dma_start(out=identb[:], in_=ib_d[:])
            nc.sync.dma_start(out=gth0[:, G0D:2 * G0D],
                              in_=xh_d[:, G0D:2 * G0D])
            wvt = cpool.tile([128, NB * 2 * RC], F32)
            nc.sync.dma_start(out=wvt[:], in_=wv_d[:])

            rbs_hist = [None] * NB
            vtA_hist = [None] * NB
            vtb_hist = [None] * NB
            in_off = [2 * G0D]
            out_off = [0]

            def fold_block(k):
                G = BT[k]
                w = G * 2 * D
                if k == 0:
                    gth = gth0
                else:
                    gth = gpool.tile([128, w], BF16, tag="gth",
                                     name=f"gth{k}")
                    half = G * D
                    nc.sync.dma_start(
                        out=gth[:, 0:half],
                        in_=xh_d[:, in_off[0]: in_off[0] + half])
                    nc.sync.dma_start(
                        out=gth[:, half:w],
                        in_=xh_d[:, in_off[0] + half: in_off[0] + w])
                    in_off[0] += w
                rbss = []
                for chunk in range(2):
                    rb = rps.tile([128, G * 21], F32, tag=f"ribs{chunk}",
                                  name=f"ribs{chunk}_{k}")
                    for j in range(7):
                        d = j if chunk == 0 else j - 6
                        # host stores each block j-plane-major, so the rhs
                        # stream is fully contiguous (fast PE ifmap fetch)
                        rhs = _ap(gth[:], (chunk * 7 + j) * G * 21,
                                  [(1, G * 21)])
                        nc.tensor.matmul(
                            rb[:], lhsT=identb[:, 128 - d:256 - d],
                            rhs=rhs, start=(j == 0), stop=(j == 6))
                    # stage ribbons to SBUF (bf16) so the row-fold adds can
                    # run on DVE/GpSimd without the PSUM port bottleneck
                    rbst = rbsp.tile([128, G * 21], BF16, tag=f"rbs{chunk}",
                                     name=f"rbs{chunk}_{k}")
                    nc.scalar.copy(out=rbst[:], in_=rb[:])
                    rbss.append(rbst)
                rbs_hist[k] = rbss
                if k >= 2:
                    rbs_hist[k - 2] = None

            def vt_stage(k):
                vtbs = []
                for chunk in range(2):
                    # chunk 0 folds on DVE, chunk 1 on GpSimd (idle engine,
                    # SBUF-only — that's why ribbons were staged to SBUF)
                    eng = nc.vector if chunk == 0 else nc.gpsimd
                    vt = vtsbp.tile([128, RC], F32, tag=f"vt{chunk}",
                                    name=f"vt{chunk}_{k}")
                    eng.memset(vt[:], 0.0)
                    for role in (1, 0):
                        gen = k - role
                        if gen < 0:
                            continue
                        Gg = BT[gen]
                        for i in range(7):
                            rl_of_g0 = i + 6 - B * role
                            g_lo = max(0, -rl_of_g0)
                            g_hi = min(Gg - 1, (B + 5) - rl_of_g0)
                            if g_lo > g_hi:
                                continue
                            ng = g_hi - g_lo + 1
                            rl0 = rl_of_g0 + g_lo
                            dst = _ap(vt[:], rl0 * 3, [(3, ng), (1, 3)])
                            src = _ap(rbs_hist[gen][chunk][:],
                                      g_lo * 21 + i, [(21, ng), (7, 3)])
                            eng.tensor_tensor(out=dst, in0=dst, in1=src,
                                              op=ADD)
                    vtb = vtbp.tile([128, RC], BF16, tag=f"vtb{chunk}",
                                    name=f"vtb{chunk}_{k}")
                    wv = wvt[:, (k * 2 + chunk) * RC:(k * 2 + chunk + 1) * RC]
                    eng.tensor_tensor(out=vtb[:], in0=vt[:], in1=wv, op=MUL)
                    vtbs.append(vtb)
                vtb_hist[k] = vtbs

            def unfold_tail(k):
                vtbs = vtb_hist[k]
                vb = vbp.tile([RC, 256], BF16, tag="vb", name=f"vb{k}")
                for chunk in range(2):
                    nc.tensor.matmul(
                        vb[:, chunk * 128:(chunk + 1) * 128],
                        lhsT=vtbs[chunk][:, 0:RC], rhs=identb[:, 128:256],
                        is_transpose=True,
                        start=(chunk == 0), stop=(chunk == 1))
                vsb = vbs.tile([RC, 256], BF16, tag="vsb", name=f"vsb{k}")
                nc.scalar.copy(out=vsb[:], in_=vb[:])
                t_lo, t_hi = OT[k]
                nt = t_hi - t_lo + 1
                goff = t_lo - (B * k - 6)
                w = nt * 2 * D
                half = nt * D
                stg = stgp.tile([128, w], BF16, tag="stg", name=f"stg{k}")
                cpy = 0
                # ci-major stg layout: each ci half stores as soon as its 7
                # copies land. Stores go via gpsimd SWDGE: keeps them off
                # the Sync HWDGE FIFO so the next block's input load isn't
                # head-of-line blocked behind this store's dependencies
                for ci, base in ((0, 0), (1, 122)):
                    for j in range(7):
                        pj = pjp.tile([128, RC], BF16, tag="pj",
                                      name=f"pj{k}_{j}_{ci}")
                        nc.tensor.matmul(
                            pj[:], lhsT=vsb[:, base + j: base + j + 128],
                            rhs=identb[0:RC, 128:128 + RC],
                            is_transpose=True, start=True, stop=True)
                        src = _ap(pj[:], goff * 3, [(3, nt), (3, 7), (1, 3)])
                        dst = _ap(stg[:], ci * half + j * 21,
                                  [(D, nt), (1, 21)])
                        if cpy % 2 == 0:
                            nc.scalar.copy(out=dst, in_=src)
                        else:
                            nc.vector.tensor_copy(out=dst, in_=src)
                        cpy += 1
                nc.gpsimd.dma_start(out=y_d[:, out_off[0]: out_off[0] + w],
                                    in_=stg[:])
                out_off[0] += w

            for k in range(NB):
                fold_block(k)
                vt_stage(k)
                unfold_tail(k)

    nc.compile()
    return nc


_NC_CACHE = [None]


def _build_in_maps(x, nlInds):
    cores = [_host_prep_core(x, nlInds, c) for c in range(8)]
    idw = _identw()
    import ml_dtypes
    idb = idw.astype(ml_dtypes.bfloat16)
    in_maps = [dict(x_hi=cr["x_hi"], winv=cr["winv"],
                    identb=idb) for cr in cores]
    return cores, in_maps


def kernel(x, nlDists, nlInds, pixels_h, pixels_w):
    x = np.ascontiguousarray(np.asarray(x, dtype=np.float32))
    nlInds = np.asarray(nlInds)
    if _NC_CACHE[0] is None:
        _NC_CACHE[0] = build_nc()
    nc = _NC_CACHE[0]
    cores, in_maps = _build_in_maps(x, nlInds)
    res = run_bass_kernel_spmd(nc, in_maps, list(range(8)))
    out = np.zeros((T, P, 1, 147), np.float32)
    for c in range(8):
        cr = cores[c]
        y = np.asarray(res.results[c]["y_core"], np.float32)  # (128, OUT_W)
        ys = y.reshape(128, SOUT, D).transpose(1, 0, 2).reshape(-1, D)
        sidx = cr["sidx"].reshape(-1)
        valid = sidx >= 0
        out[cr["f"], cr["sel"][sidx[valid]], 0] = ys[valid][:, COLPERM]
    return out


# revision 28
# speedup vs baseline: 1.1671x; 1.0960x over previous
"""Trainium2 Bass kernel for nn_Aggregation0 (scatter_memory).

8 cores = 4 frames x 2 image-halves (SPMD, one program). The host pre-sorts
patch rows into destination order per core (the sharding/permutation step);
the device does the full aggregation compute with sequential DMA:
  fold: bf16 matmuls vs shifted-identity weights accumulate DIRECTLY into
  per-block row ribbons in PSUM (dst AP aliases columns at 3*(g+i)+c, a
  zero-init matmul arms the has_written bits so aliased writes accumulate)
  -> DVE seam-join + 1/w normalize (cast to bf16)
  -> bf16 PE transpose to (row,ch)-major -> transpose-mode unfold per column
  shift -> strided assembly copies (bf16) -> sequential store (host
  inverse-maps). All HBM traffic is bf16 (tolerance 2e-2 >> bf16 roundoff).
"""
import sys
if '/opt/trn_rl_repo' not in sys.path:
    sys.path.insert(0, '/opt/trn_rl_repo')
import numpy as np

import concourse.bacc as bacc
import concourse.bass as bass
import concourse.mybir as mybir
import concourse.tile as tile
from concourse.bass_utils import run_bass_kernel_spmd

T, HP, WP = 4, 256, 256
PS = 7
NPOS = 250
P = NPOS * NPOS
B = 24                   # tops per block
NB = 6
BT = [B] * 5 + [11]
RC = 3 * (B + 6)         # rowtile (r,ch) columns = 90
GW = 131
NREAL = GW * NPOS        # 32750
F32 = mybir.dt.float32
BF16 = mybir.dt.bfloat16
D = 147
# out-tops per block k: [Bk-6, Bk+B-7] clipped to [0, 130]
OT = [(max(0, B * k - 6), min(130, B * k + B - 7)) for k in range(NB)]
SIN = sum(BT) * 2            # 262 input slots of [128, 147]
SOUT = sum(t[1] - t[0] + 1 for t in OT) * 2   # 262 output slots
IN_W = SIN * D
OUT_W = SOUT * D


# device rows use (j, i, ch) element order; COLPERM maps back to (ch, i, j)
COLPERM = np.zeros(147, np.int64)
for _c in range(3):
    for _i in range(7):
        for _j in range(7):
            COLPERM[_c * 49 + _i * 7 + _j] = _j * 21 + _i * 3 + _c


def _cntf(z):
    z = np.asarray(z, np.float64)
    return np.minimum(6, z) - np.maximum(0, z - 249) + 1


def _host_prep_core(x, nlInds, c):
    f, h = c >> 1, c & 1
    g0 = 0 if h == 0 else 119
    o_lo, o_hi = (0, 124) if h == 0 else (6, 130)
    inds = nlInds[f, :, 0]
    top = inds[:, 1].astype(np.int64)
    left = inds[:, 2].astype(np.int64)
    invperm = np.empty(P, np.int64)
    invperm[top * NPOS + left] = np.arange(P)
    sel = np.nonzero((top >= g0) & (top <= g0 + 130))[0]
    rank = np.full(P, -1, np.int64)
    rank[sel] = np.arange(NREAL)
    ar128 = np.arange(128)
    ZR, DM = NREAL, -1
    gidx = np.zeros((SIN, 128), np.int64)
    s = 0
    for k in range(NB):
        for g in range(BT[k]):
            gt_ = g0 + B * k + g
            for ci, base in ((0, 0), (1, 122)):
                gidx[s] = rank[invperm[gt_ * NPOS + base + ar128]]
                s += 1
    sidx = np.full((SOUT, 128), DM, np.int64)
    s = 0
    for k in range(NB):
        t_lo, t_hi = OT[k]
        # ci-major slot order (matches device stg layout / half-DMA split)
        for ci, base in ((0, 0), (1, 122)):
            for lt in range(t_lo, t_hi + 1):
                gt_ = g0 + lt
                if o_lo <= lt <= o_hi:
                    col = rank[invperm[gt_ * NPOS + base + ar128]]
                    sidx[s] = col
                    if ci == 1:
                        sidx[s, :6] = DM
                s += 1
    # x plane: (128, SIN*147) dest-ordered slots, single bf16 plane
    # (rel-err tolerance 2e-2 >> bf16 roundoff ~4e-3)
    import ml_dtypes
    xp = np.concatenate([x[f, sel, 0], np.zeros((1, D), np.float32)], axis=0)
    xs = xp[gidx.reshape(-1)].reshape(SIN, 128, D).transpose(1, 0, 2) \
        .reshape(128, IN_W)
    # reorder each block's slab (g, ci, c, i, j) -> (ci, j, g, c, i) so the
    # device fold matmul's rhs stream is contiguous per (chunk, j)
    off = 0
    for k in range(NB):
        G = BT[k]
        w = G * 2 * D
        slab = xs[:, off:off + w].reshape(128, G, 2, 3, 7, 7)
        xs[:, off:off + w] = np.ascontiguousarray(
            slab.transpose(0, 2, 5, 1, 3, 4)).reshape(128, w)
        off += w
    x_hi = np.ascontiguousarray(xs.astype(ml_dtypes.bfloat16))
    # winv per (block, chunk): [128(par c), 90(r,ch)]
    winv = np.zeros((NB, 2, 128, RC), np.float32)
    for k in range(NB):
        for rl in range(B + 6):
            gr = g0 + B * k - 6 + rl
            lr = B * k - 6 + rl
            if 0 <= gr <= 255 and 0 <= lr <= 136:
                wr = _cntf(gr)
                for chunk in range(2):
                    cs = chunk * 128 + np.arange(128)
                    winv[k, chunk, :, rl * 3:rl * 3 + 3] = \
                        (1.0 / (wr * _cntf(cs)))[:, None]
    return dict(x_hi=x_hi,
                winv=np.ascontiguousarray(
                    winv.transpose(2, 0, 1, 3).reshape(128, NB * 2 * RC)),
                f=f, sel=sel, sidx=sidx)


def _identw():
    w = np.zeros((128, 262), np.float32)
    w[np.arange(128), np.arange(128) + 128] = 1.0
    return w


def _ap(base, off, dims):
    return bass.AP(base.tensor, base.offset + off,
                   [list(base.ap[0])] + [list(d) for d in dims])


def build_nc():
    nc = bacc.Bacc("TRN2", target_bir_lowering=False, debug=False, num_devices=8)
    xh_d = nc.declare_dram_parameter("x_hi", [128, IN_W], BF16, isOutput=False)
    ib_d = nc.declare_dram_parameter("identb", [128, 262], BF16, isOutput=False)
    wv_d = nc.declare_dram_parameter("winv", [128, NB * 2 * RC], F32,
                                     isOutput=False)
    y_d = nc.declare_dram_parameter("y_core", [128, OUT_W], BF16, isOutput=True)

    ADD = mybir.AluOpType.add
    MUL = mybir.AluOpType.mult

    with tile.TileContext(nc) as tc:
        with tc.tile_pool(name="const", bufs=1) as cpool, \
             tc.tile_pool(name="gp", bufs=3) as gpool, \
             tc.tile_pool(name="rps", bufs=2, space="PSUM") as rps, \
             tc.tile_pool(name="rbs", bufs=3) as rbsp, \
             tc.tile_pool(name="vtsb", bufs=2) as vtsbp, \
             tc.tile_pool(name="vtbp", bufs=3) as vtbp, \
             tc.tile_pool(name="vbp", bufs=1, space="PSUM") as vbp, \
             tc.tile_pool(name="vbs", bufs=2) as vbs, \
             tc.tile_pool(name="pjp", bufs=3, space="PSUM") as pjp, \
             tc.tile_pool(name="stg", bufs=3) as stgp:
            # block 0's chunk-0 half is issued before the consts so the
            # first fold can start ~3us in; each block's input lands as two
            # per-chunk half-DMAs (chunk-0 folds depend only on the first)
            G0D = BT[0] * D
            gth0 = gpool.tile([128, 2 * G0D], BF16, tag="gth", name="gth0")
            nc.sync.dma_start(out=gth0[:, 0:G0D], in_=xh_d[:, 0:G0D])
            identb = cpool.tile([128, 262], BF16)
            nc.sync.dma_start(out=identb[:], in_=ib_d[:])
            nc.sync.dma_start(out=gth0[:, G0D:2 * G0D],
                              in_=xh_d[:, G0D:2 * G0D])
            wvt = cpool.tile([128, NB * 2 * RC], F32)
            nc.sync.dma_start(out=wvt[:], in_=wv_d[:])

            rbs_hist = [None] * NB
            vtA_hist = [None] * NB
            vtb_hist = [None] * NB
            in_off = [2 * G0D]
            out_off = [0]

            def fold_block(k):
                G = BT[k]
                w = G * 2 * D
                if k == 0:
                    gth = gth0
                else:
                    gth = gpool.tile([128, w], BF16, tag="gth",
                                     name=f"gth{k}")
                    half = G * D
                    nc.sync.dma_start(
                        out=gth[:, 0:half],
                        in_=xh_d[:, in_off[0]: in_off[0] + half])
                    nc.sync.dma_start(
                        out=gth[:, half:w],
                        in_=xh_d[:, in_off[0] + half: in_off[0] + w])
                    in_off[0] += w
                rbss = []
                for chunk in range(2):
                    rb = rps.tile([128, G * 21], F32, tag=f"ribs{chunk}",
                                  name=f"ribs{chunk}_{k}")
                    for j in range(7):
                        d = j if chunk == 0 else j - 6
                        # host stores each block j-plane-major, so the rhs
                        # stream is fully contiguous (fast PE ifmap fetch)
                        rhs = _ap(gth[:], (chunk * 7 + j) * G * 21,
                                  [(1, G * 21)])
                        nc.tensor.matmul(
                            rb[:], lhsT=identb[:, 128 - d:256 - d],
                            rhs=rhs, start=(j == 0), stop=(j == 6))
                    # stage ribbons to SBUF (bf16) so the row-fold adds can
                    # run on DVE/GpSimd without the PSUM port bottleneck
                    rbst = rbsp.tile([128, G * 21], BF16, tag=f"rbs{chunk}",
                                     name=f"rbs{chunk}_{k}")
                    nc.scalar.copy(out=rbst[:], in_=rb[:])
                    rbss.append(rbst)
                rbs_hist[k] = rbss
                if k >= 2:
                    rbs_hist[k - 2] = None

            def vt_stage(k):
                vtbs = []
                for chunk in range(2):
                    # chunk 0 folds on DVE, chunk 1 on GpSimd (idle engine,
                    # SBUF-only — that's why ribbons were staged to SBUF)
                    eng = nc.vector if chunk == 0 else nc.gpsimd
                    vt = vtsbp.tile([128, RC], F32, tag=f"vt{chunk}",
                                    name=f"vt{chunk}_{k}")
                    eng.memset(vt[:], 0.0)
                    for role in (1, 0):
                        gen = k - role
                        if gen < 0:
                            continue
                        Gg = BT[gen]
                        for i in range(7):
                            rl_of_g0 = i + 6 - B * role
                            g_lo = max(0, -rl_of_g0)
                            g_hi = min(Gg - 1, (B + 5) - rl_of_g0)
                            if g_lo > g_hi:
                                continue
                            ng = g_hi - g_lo + 1
                            rl0 = rl_of_g0 + g_lo
                            dst = _ap(vt[:], rl0 * 3, [(3, ng), (1, 3)])
                            src = _ap(rbs_hist[gen][chunk][:],
                                      g_lo * 21 + i, [(21, ng), (7, 3)])
                            eng.tensor_tensor(out=dst, in0=dst, in1=src,
                                              op=ADD)
                    vtb = vtbp.tile([128, RC], BF16, tag=f"vtb{chunk}",
                                    name=f"vtb{chunk}_{k}")
                    wv = wvt[:, (k * 2 + chunk) * RC:(k * 2 + chunk + 1) * RC]
                    eng.tensor_tensor(out=vtb[:], in0=vt[:], in1=wv, op=MUL)
                    vtbs.append(vtb)
                vtb_hist[k] = vtbs

            def unfold_tail(k):
                vtbs = vtb_hist[k]
                vb = vbp.tile([RC, 256], BF16, tag="vb", name=f"vb{k}")
                for chunk in range(2):
                    nc.tensor.matmul(
                        vb[:, chunk * 128:(chunk + 1) * 128],
                        lhsT=vtbs[chunk][:, 0:RC], rhs=identb[:, 128:256],
                        is_transpose=True,
                        start=(chunk == 0), stop=(chunk == 1))
                vsb = vbs.tile([RC, 256], BF16, tag="vsb", name=f"vsb{k}")
                nc.scalar.copy(out=vsb[:], in_=vb[:])
                t_lo, t_hi = OT[k]
                nt = t_hi - t_lo + 1
                goff = t_lo - (B * k - 6)
                w = nt * 2 * D
                half = nt * D
                stg = stgp.tile([128, w], BF16, tag="stg", name=f"stg{k}")
                cpy = 0
                # ci-major stg layout: each ci half stores as soon as its 7
                # copies land. Stores go via gpsimd SWDGE: keeps them off
                # the Sync HWDGE FIFO so the next block's input load isn't
                # head-of-line blocked behind this store's dependencies
                for ci, base in ((0, 0), (1, 122)):
                    for j in range(7):
                        pj = pjp.tile([128, RC], BF16, tag="pj",
                                      name=f"pj{k}_{j}_{ci}")
                        nc.tensor.matmul(
                            pj[:], lhsT=vsb[:, base + j: base + j + 128],
                            rhs=identb[0:RC, 128:128 + RC],
                            is_transpose=True, start=True, stop=True)
                        src = _ap(pj[:], goff * 3, [(3, nt), (3, 7), (1, 3)])
                        dst = _ap(stg[:], ci * half + j * 21,
                                  [(D, nt), (1, 21)])
                        if cpy % 2 == 0:
                            nc.scalar.copy(out=dst, in_=src)
                        else:
                            nc.vector.tensor_copy(out=dst, in_=src)
                        cpy += 1
                nc.gpsimd.dma_start(out=y_d[:, out_off[0]: out_off[0] + w],
                                    in_=stg[:])
                out_off[0] += w

            # pipeline-shift: block k's vb/pj/copy/store chain is issued
            # after block k+1's fold, so the PE never stalls waiting for
            # the DVE/ACT row-fold roundtrip of the block it just folded
            for k in range(NB):
                fold_block(k)
                vt_stage(k)
                if k >= 1:
                    unfold_tail(k - 1)
            unfold_tail(NB - 1)

    nc.compile()
    return nc


_NC_CACHE = [None]


def _build_in_maps(x, nlInds):
    cores = [_host_prep_core(x, nlInds, c) for c in range(8)]
    idw = _identw()
    import ml_dtypes
    idb = idw.astype(ml_dtypes.bfloat16)
    in_maps = [dict(x_hi=cr["x_hi"], winv=cr["winv"],
                    identb=idb) for cr in cores]
    return cores, in_maps


def kernel(x, nlDists, nlInds, pixels_h, pixels_w):
    x = np.ascontiguousarray(np.asarray(x, dtype=np.float32))
    nlInds = np.asarray(nlInds)
    if _NC_CACHE[0] is None:
        _NC_CACHE[0] = build_nc()
    nc = _NC_CACHE[0]
    cores, in_maps = _build_in_maps(x, nlInds)
    res = run_bass_kernel_spmd(nc, in_maps, list(range(8)))
    out = np.zeros((T, P, 1, 147), np.float32)
    for c in range(8):
        cr = cores[c]
        y = np.asarray(res.results[c]["y_core"], np.float32)  # (128, OUT_W)
        ys = y.reshape(128, SOUT, D).transpose(1, 0, 2).reshape(-1, D)
        sidx = cr["sidx"].reshape(-1)
        valid = sidx >= 0
        out[cr["f"], cr["sel"][sidx[valid]], 0] = ys[valid][:, COLPERM]
    return out
